# revision 12
# baseline (speedup 1.0000x reference)
"""Trainium2 Bass kernel for BlockAxialDown (maxpool + axial attention + 1x1 conv + batchnorm).

Contract: kernel(**inputs) takes FULL unsharded inputs, returns FULL output.
Sharding: data-parallel over batch B=8 across 8 NeuronCores (1 image/core);
BatchNorm batch stats combined with a tiny (128,4) AllReduce; weights replicated.
Matmul operands in bf16 (fp32 PSUM accumulation), everything else fp32.

v2: attention-weight transposes on the tensor engine (matmul vs identity)
instead of serialized DMA-xbar transposes; elementwise softmax work batched
into [128,512] ops; conv single-pass with y kept in SBUF bf16.
"""

import sys

import numpy as np

for _p in ("/opt/trn_rl_repo", "/root/.axon_site/_ro/trn_rl_repo"):
    if _p not in sys.path:
        sys.path.append(_p)

B, C, H, W = 8, 128, 256, 256
H2, W2 = 128, 128
E = 2 * C
NPOS = H2 * W2
NCORES = 8
BN_EPS = 1e-5
DH = C // 2
SCALE = DH ** -0.5

_CACHE = {}


def _build_program():
    import concourse.tile as tile
    from concourse import bacc, mybir
    from concourse.alu_op_type import AluOpType
    from contextlib import ExitStack

    F32 = mybir.dt.float32
    BF16 = mybir.dt.bfloat16
    AF = mybir.ActivationFunctionType
    AX = mybir.AxisListType
    P = 128

    nc = bacc.Bacc("TRN2", target_bir_lowering=False, debug=False, num_devices=NCORES)

    # ---- DRAM I/O ----
    x_d = nc.dram_tensor("x", [C, H, W], BF16, kind="ExternalInput").ap()
    wq_w_d = nc.dram_tensor("wq_w", [C, C], BF16, kind="ExternalInput").ap()
    wk_w_d = nc.dram_tensor("wk_w", [C, C], BF16, kind="ExternalInput").ap()
    wv_w_d = nc.dram_tensor("wv_w", [C, C], BF16, kind="ExternalInput").ap()
    wo_w_d = nc.dram_tensor("wo_w", [C, C], BF16, kind="ExternalInput").ap()
    wq_h_d = nc.dram_tensor("wq_h", [C, C], BF16, kind="ExternalInput").ap()
    wk_h_d = nc.dram_tensor("wk_h", [C, C], BF16, kind="ExternalInput").ap()
    wv_h_d = nc.dram_tensor("wv_h", [C, C], BF16, kind="ExternalInput").ap()
    wo_h_d = nc.dram_tensor("wo_h", [C, C], BF16, kind="ExternalInput").ap()
    bsum_d = nc.dram_tensor("bsum", [C, 1], F32, kind="ExternalInput").ap()
    ident_d = nc.dram_tensor("ident", [128, 128], BF16, kind="ExternalInput").ap()
    convA_d = nc.dram_tensor("convA", [C, E], BF16, kind="ExternalInput").ap()
    convX_d = nc.dram_tensor("convX", [C, E], BF16, kind="ExternalInput").ap()
    gamma2_d = nc.dram_tensor("gamma2", [C, 2], F32, kind="ExternalInput").ap()
    beta2_d = nc.dram_tensor("beta2", [C, 2], F32, kind="ExternalInput").ap()
    out_d = nc.dram_tensor("out", [E, H2, W2], F32, kind="ExternalOutput").ap()
    stats_in_d = nc.dram_tensor("stats_in", [P, 4], F32).ap()
    stats_out_d = nc.dram_tensor("stats_out", [P, 4], F32, addr_space="Shared").ap()

    with tile.TileContext(nc) as tc, ExitStack() as ctx:
        const = ctx.enter_context(tc.tile_pool(name="const", bufs=1))
        cube = ctx.enter_context(tc.tile_pool(name="cube", bufs=1))
        stage = ctx.enter_context(tc.tile_pool(name="stage", bufs=3))
        work = ctx.enter_context(tc.tile_pool(name="work", bufs=2))
        stats = ctx.enter_context(tc.tile_pool(name="stats", bufs=1))
        psum = ctx.enter_context(tc.tile_pool(name="psum", bufs=1, space="PSUM"))

        # ---- constants ----
        def cload(name, ap_d, shape, dt):
            t = const.tile(shape, dt, name=name)
            nc.sync.dma_start(out=t[:], in_=ap_d)
            return t

        wq_w = cload("wq_w_t", wq_w_d, [C, C], BF16)
        wk_w = cload("wk_w_t", wk_w_d, [C, C], BF16)
        wv_w = cload("wv_w_t", wv_w_d, [C, C], BF16)
        wo_w = cload("wo_w_t", wo_w_d, [C, C], BF16)
        wq_h = cload("wq_h_t", wq_h_d, [C, C], BF16)
        wk_h = cload("wk_h_t", wk_h_d, [C, C], BF16)
        wv_h = cload("wv_h_t", wv_h_d, [C, C], BF16)
        wo_h = cload("wo_h_t", wo_h_d, [C, C], BF16)
        bsum = cload("bsum_t", bsum_d, [C, 1], F32)
        ident = cload("ident_t", ident_d, [128, 128], BF16)
        convA = cload("convA_t", convA_d, [C, E], BF16)
        convX = cload("convX_t", convX_d, [C, E], BF16)
        gamma2 = cload("gamma2_t", gamma2_d, [C, 2], F32)
        beta2 = cload("beta2_t", beta2_d, [C, 2], F32)

        xp = cube.tile([P, H2, W2], BF16)   # pooled input, channels on partitions
        acc = cube.tile([P, H2, W2], BF16)  # attention output accumulator
        y0 = cube.tile([P, NPOS], BF16)     # conv+relu output, channels 0:128
        y1 = cube.tile([P, NPOS], BF16)     # conv+relu output, channels 128:256
        xp_f = xp[:].rearrange("c h w -> c (h w)")
        acc_f = acc[:].rearrange("c h w -> c (h w)")

        # ---- phase 1: load + 2x2 maxpool ----
        xv = x_d.rearrange("c (n r) w -> c n r w", r=8)
        for i in range(H // 8):
            xin = stage.tile([P, 8, W], BF16, tag="xin")
            nc.sync.dma_start(out=xin[:], in_=xv[:, i])
            t = stage.tile([P, 8, W2], BF16, tag="wmax")
            xin4 = xin[:].rearrange("c r (w two) -> c r w two", two=2)
            nc.vector.tensor_max(t[:], xin4[:, :, :, 0], xin4[:, :, :, 1])
            t4 = t[:].rearrange("c (r2 two) w -> c r2 two w", two=2)
            nc.vector.tensor_max(xp[:, 4 * i:4 * i + 4, :], t4[:, :, 0, :], t4[:, :, 1, :])

        # ---- axial attention over a group of 4 slices ----
        # PSUM bank rule: matmuls the PE packs (disjoint row/col groups) must
        # write different banks. dots pairs (h0 rows 0:64 || h1 rows 64:128)
        # -> head-major [128,1024]: bank A = h0 chunks, bank B = h1 chunks.
        # AV pairs (h0 cols 0:64 || h1 cols 64:128) -> og0/og1 separate banks.
        def attn_group(rhs_g, slice_lhs, wq, wk, wv, wo):
            qg_ps = psum.tile([P, 512], F32, tag="proj", bufs=2, name="qg_ps")
            nc.tensor.matmul(qg_ps[:], lhsT=wq[:], rhs=rhs_g, start=True, stop=True)
            kg_ps = psum.tile([P, 512], F32, tag="proj", bufs=2, name="kg_ps")
            nc.tensor.matmul(kg_ps[:], lhsT=wk[:], rhs=rhs_g, start=True, stop=True)
            qg = work.tile([P, 512], BF16, tag="qg", bufs=2)
            nc.vector.tensor_copy(qg[:], qg_ps[:])
            kg = work.tile([P, 512], BF16, tag="kg", bufs=2)
            nc.scalar.copy(kg[:], kg_ps[:])

            vT_ps = psum.tile([P, 512], F32, tag="av", bufs=2, name="vT_ps")
            for s in range(4):
                nc.tensor.matmul(vT_ps[:, 128 * s:128 * s + 128], lhsT=slice_lhs(s),
                                 rhs=wv[:], start=True, stop=True)
            vT = work.tile([P, 512], BF16, tag="vT_sb", bufs=2)
            nc.vector.tensor_copy(vT[:], vT_ps[:])

            # dots, head-major: chunk m = h*4+s at cols 128*m
            dots_ps = psum.tile([P, 1024], F32, tag="dots", bufs=1, name="dots_ps")
            for s in range(4):
                cs = slice(128 * s, 128 * s + 128)
                nc.tensor.matmul(dots_ps[:, 128 * s:128 * s + 128],
                                 lhsT=qg[0:64, cs], rhs=kg[0:64, cs],
                                 start=True, stop=True)
                nc.tensor.matmul(dots_ps[:, 512 + 128 * s:512 + 128 * s + 128],
                                 lhsT=qg[64:128, cs], rhs=kg[64:128, cs],
                                 start=True, stop=True)
            e = work.tile([P, 1024], BF16, tag="e", bufs=2)
            nc.scalar.activation(e[:, 0:512], dots_ps[:, 0:512], AF.Exp, scale=SCALE)
            nc.scalar.activation(e[:, 512:1024], dots_ps[:, 512:1024], AF.Exp, scale=SCALE)
            sums = work.tile([P, 8], F32, tag="sums", bufs=2)
            e8 = e[:].rearrange("c (m j) -> c m j", j=128)
            nc.vector.reduce_sum(sums[:], e8, axis=AX.X)
            rcp = work.tile([P, 8], F32, tag="rcp", bufs=2)
            nc.vector.reciprocal(rcp[:], sums[:])
            for m in range(8):
                col = slice(128 * m, 128 * m + 128)
                rj = rcp[:, m:m + 1]
                if m % 2 == 0:
                    nc.gpsimd.tensor_scalar_mul(e[:, col], e[:, col], rj)
                else:
                    nc.vector.tensor_scalar_mul(e[:, col], e[:, col], rj)
            # transposes: full-array sequential matmuls, bank sharing OK
            eT_ps = psum.tile([P, 1024], F32, tag="eT", bufs=1, name="eT_ps")
            for m in range(8):
                col = slice(128 * m, 128 * m + 128)
                nc.tensor.matmul(eT_ps[:, col], lhsT=e[:, col], rhs=ident[:],
                                 start=True, stop=True)
            eT = work.tile([P, 1024], BF16, tag="eT_sb", bufs=2)
            nc.vector.tensor_copy(eT[:, 0:512], eT_ps[:, 0:512])
            nc.scalar.copy(eT[:, 512:1024], eT_ps[:, 512:1024])
            # AV: h0 -> og0 partitions 0:64, h1 -> og1 partitions 64:128
            og0 = psum.tile([P, 512], F32, tag="av", bufs=2, name="og0")
            og1 = psum.tile([P, 512], F32, tag="av", bufs=2, name="og1")
            for s in range(4):
                oc = slice(128 * s, 128 * s + 128)
                nc.tensor.matmul(og0[0:64, oc],
                                 lhsT=vT[:, 128 * s:128 * s + 64],
                                 rhs=eT[:, 128 * s:128 * s + 128],
                                 start=True, stop=True)
                nc.tensor.matmul(og1[64:128, oc],
                                 lhsT=vT[:, 128 * s + 64:128 * s + 128],
                                 rhs=eT[:, 512 + 128 * s:512 + 128 * s + 128],
                                 start=True, stop=True, tile_position=(0, 64))
            og = work.tile([P, 512], BF16, tag="og_sb", bufs=2)
            nc.scalar.copy(og[0:64, :], og0[0:64, :])
            nc.vector.tensor_copy(og[64:128, :], og1[64:128, :])
            yg_ps = psum.tile([P, 512], F32, tag="proj", bufs=2, name="yg_ps")
            nc.tensor.matmul(yg_ps[:], lhsT=wo[:], rhs=og[:], start=True, stop=True)
            return yg_ps

        # ---- phase 2: W-direction attention (rows contiguous) ----
        for g in range(H2 // 4):
            rhs_g = xp[:, 4 * g:4 * g + 4, :]
            yg = attn_group(rhs_g, lambda s, g=g: xp[:, 4 * g + s, :],
                            wq_w, wk_w, wv_w, wo_w)
            nc.scalar.activation(acc_f[:, 512 * g:512 * (g + 1)], yg[:],
                                 AF.Identity, bias=bsum[:, 0:1], scale=1.0)

        # ---- phase 3: H-direction attention (columns, strided) ----
        for g in range(W2 // 4):
            rhs_g = xp[:, :, 4 * g:4 * g + 4].rearrange("c h w -> c w h")
            yg = attn_group(rhs_g, lambda s, g=g: xp[:, :, 4 * g + s],
                            wq_h, wk_h, wv_h, wo_h)
            acc_sl = acc[:, :, 4 * g:4 * g + 4]
            yg_r = yg[:].rearrange("c (s i) -> c i s", s=4)
            nc.vector.tensor_add(acc_sl, acc_sl, yg_r)

        # ---- phase 3.5: relu over acc ----
        for j in range(4):
            sl = acc_f[:, 4096 * j:4096 * (j + 1)]
            nc.vector.tensor_scalar_max(sl, sl, 0.0)

        # ---- phase 4: conv + relu (single pass, y kept in SBUF) + stats ----
        bnb = [stats.tile([P, 32, 6], F32, name=f"bnb{i}") for i in range(2)]
        ys = [y0, y1]
        for p in range(NPOS // 512):
            pos = slice(512 * p, 512 * (p + 1))
            for eh in range(2):
                yps = psum.tile([P, 512], F32, tag="proj", bufs=2, name=f"conv_ps{eh}")
                ce = slice(128 * eh, 128 * eh + 128)
                nc.tensor.matmul(yps[:], lhsT=convA[:, ce], rhs=acc_f[:, pos],
                                 start=True, stop=False)
                nc.tensor.matmul(yps[:], lhsT=convX[:, ce], rhs=xp_f[:, pos],
                                 start=False, stop=True)
                yr = work.tile([P, 512], F32, tag=f"yr{eh}")
                nc.scalar.activation(yr[:], yps[:], AF.Relu)
                nc.vector.bn_stats(bnb[eh][:, p, :], yr[:])
                nc.vector.tensor_copy(ys[eh][:, pos], yr[:])

        mv = stats.tile([P, 2, 2], F32)
        for eh in range(2):
            nc.vector.bn_aggr(mv[:, eh, :], bnb[eh][:])
        cc_in = stats.tile([P, 4], F32)
        for eh in range(2):
            # [mean, E[y^2]] per half; E[y^2] = var + mean^2
            nc.vector.tensor_copy(cc_in[:, 2 * eh:2 * eh + 1], mv[:, eh, 0:1])
            nc.vector.scalar_tensor_tensor(
                cc_in[:, 2 * eh + 1:2 * eh + 2],
                in0=mv[:, eh, 0:1], scalar=mv[:, eh, 0:1], in1=mv[:, eh, 1:2],
                op0=AluOpType.mult, op1=AluOpType.add)
        nc.sync.dma_start(out=stats_in_d, in_=cc_in[:])
        nc.gpsimd.collective_compute(
            "AllReduce", AluOpType.add,
            replica_groups=[list(range(NCORES))],
            ins=[stats_in_d], outs=[stats_out_d])
        gst = stats.tile([P, 4], F32)
        nc.sync.dma_start(out=gst[:], in_=stats_out_d)

        # ---- phase 5: BN affine coefficients ----
        t0 = stats.tile([P, 4], F32)
        nc.vector.tensor_scalar_mul(t0[:], gst[:], 1.0 / NCORES)
        t0v = t0[:].rearrange("c (e two) -> c e two", two=2)
        m2 = stats.tile([P, 2], F32)
        veps = stats.tile([P, 2], F32)
        for eh in range(2):
            nc.vector.tensor_mul(m2[:, eh:eh + 1], t0v[:, eh, 0:1], t0v[:, eh, 0:1])
            nc.vector.scalar_tensor_tensor(
                veps[:, eh:eh + 1],
                in0=t0v[:, eh, 1:2], scalar=BN_EPS, in1=m2[:, eh:eh + 1],
                op0=AluOpType.add, op1=AluOpType.subtract)
        sd = stats.tile([P, 2], F32)
        nc.scalar.sqrt(sd[:], veps[:])
        rstd = stats.tile([P, 2], F32)
        nc.vector.reciprocal(rstd[:], sd[:])
        scl = stats.tile([P, 2], F32)
        nc.vector.tensor_mul(scl[:], gamma2[:], rstd[:])
        msc = stats.tile([P, 2], F32)
        means = stats.tile([P, 2], F32)
        nc.vector.tensor_copy(means[:, 0:1], t0v[:, 0, 0:1])
        nc.vector.tensor_copy(means[:, 1:2], t0v[:, 1, 0:1])
        nc.vector.tensor_mul(msc[:], means[:], scl[:])
        shift = stats.tile([P, 2], F32)
        nc.vector.tensor_sub(shift[:], beta2[:], msc[:])

        # ---- phase 6: affine + output ----
        out_r = out_d.rearrange("(two c) h w -> two c (h w)", two=2)
        for p in range(NPOS // 512):
            pos = slice(512 * p, 512 * (p + 1))
            for eh in range(2):
                yo = work.tile([P, 512], F32, tag=f"yo{eh}", bufs=3)
                nc.vector.tensor_scalar(
                    yo[:], ys[eh][:, pos], scl[:, eh:eh + 1], shift[:, eh:eh + 1],
                    op0=AluOpType.mult, op1=AluOpType.add)
                nc.sync.dma_start(out=out_r[eh, :, pos], in_=yo[:])

    nc.finalize()
    return nc


def _get_program():
    if "nc" not in _CACHE:
        _CACHE["nc"] = _build_program()
    return _CACHE["nc"]


def _make_in_maps(x, Wq_h, Wkv_h, Wout_h, bout_h, Wq_w, Wkv_w, Wout_w, bout_w,
                  conv_w, gamma, beta):
    import ml_dtypes
    f = np.float32
    bf = ml_dtypes.bfloat16
    shared = {
        "wq_w": np.ascontiguousarray(np.asarray(Wq_w, f).astype(bf)),
        "wk_w": np.ascontiguousarray(np.asarray(Wkv_w, f)[:, :C].astype(bf)),
        "wv_w": np.ascontiguousarray(np.asarray(Wkv_w, f)[:, C:].astype(bf)),
        "wo_w": np.ascontiguousarray(np.asarray(Wout_w, f).astype(bf)),
        "wq_h": np.ascontiguousarray(np.asarray(Wq_h, f).astype(bf)),
        "wk_h": np.ascontiguousarray(np.asarray(Wkv_h, f)[:, :C].astype(bf)),
        "wv_h": np.ascontiguousarray(np.asarray(Wkv_h, f)[:, C:].astype(bf)),
        "wo_h": np.ascontiguousarray(np.asarray(Wout_h, f).astype(bf)),
        "bsum": np.ascontiguousarray((np.asarray(bout_h, f) + np.asarray(bout_w, f)).reshape(C, 1)),
        "ident": np.ascontiguousarray(np.eye(128, dtype=f).astype(bf)),
        "convA": np.ascontiguousarray(np.asarray(conv_w, f)[:C, :].astype(bf)),
        "convX": np.ascontiguousarray(np.asarray(conv_w, f)[C:, :].astype(bf)),
        "gamma2": np.ascontiguousarray(np.asarray(gamma, f).reshape(2, C).T),
        "beta2": np.ascontiguousarray(np.asarray(beta, f).reshape(2, C).T),
    }
    xb = np.asarray(x, f).astype(bf)
    return [{**shared, "x": np.ascontiguousarray(xb[b])} for b in range(B)]


def run(trace=False, **inputs):
    from concourse.bass_utils import run_bass_kernel_spmd

    nc = _get_program()
    in_maps = _make_in_maps(**inputs)
    res = run_bass_kernel_spmd(nc, in_maps, list(range(NCORES)), trace=trace)
    out = np.stack([res.results[b]["out"] for b in range(B)], axis=0)
    return out, res


def kernel(**inputs):
    out, _ = run(trace=False, **inputs)
    return out
